# revision 22
# baseline (speedup 1.0000x reference)
"""Trainium2 Bass kernel for nn_MaxPool_Agg (GNN max-pool aggregation).

Reference computation (per node n, fanout K):
    h   = x[neigh]                      # [N, K, F_IN] gather
    h   = relu(h @ W + b)               # linear + relu on each neighbor
    out = max(h, axis=1)                # elementwise max-pool over K

Key algebraic restructure: the row gather commutes with the (row-wise)
linear map, and relu(v + b) is monotone in v, so

    out[n] = relu( max_k (x @ W)[neigh[n, k]] + b )

We therefore:
  phase 1: compute the dense table  zlin = x @ W   (25000 x 256, bf16)
           once per core (3.3 GFLOP instead of the naive 105 GFLOP),
  phase 2: row-gather zlin[neigh] straight from HBM with the SWDGE
           dma_gather instruction (512B bf16 rows), max-tree over K=32
           on the DVE, then + bias and relu on the pooled result.

Sharding: nodes (rows of neigh / out) are split evenly across the 8
NeuronCores; x, W, b are replicated (each core redundantly computes the
full zlin table, which is cheaper than an all-gather at this size).

Host-side prep: x is shipped pre-transposed in bf16 ([256, N] padded to
a multiple of 128 columns) so the matmul's stationary operand loads
directly; neigh is remapped to the interleaved row order the on-device
z-table eviction DMA produces, wrapped into the [16, n/16] index layout
dma_gather expects, and converted to int16.
"""

import numpy as np
import ml_dtypes

import concourse.bass as bass  # noqa: F401  (registers engine classes)
import concourse.mybir as mybir
import concourse.tile as tile
from concourse import bacc, bass_utils

BF16 = mybir.dt.bfloat16
F32 = mybir.dt.float32
I16 = mybir.dt.int16


class Cfg:
    def __init__(self, n=25000, k=32, f=256, n_cores=8, tiles_per_slab=14,
                 blk=128, gather_ni=1024, single_packet=False, nq=4,
                 prep_ahead=6):
        self.N = n
        self.K = k
        self.F = f
        self.NCORES = n_cores
        self.NPC = n // n_cores             # nodes per core
        self.NTILES = -(-n // 128)          # z tiles of 128 rows
        self.TPS = tiles_per_slab
        assert self.NTILES % self.TPS == 0, (self.NTILES, self.TPS)
        self.NSLABS = self.NTILES // self.TPS
        self.NPAD = self.NTILES * 128       # padded z row count
        self.BLK = blk                      # output nodes per gather block
        self.NBLK = -(-self.NPC // blk)
        self.IDX_PER_BLK = blk * k
        self.IDX_COLS = self.NBLK * self.IDX_PER_BLK // 16
        # idx per dma_gather call (4096 + single_packet=True fails on HW)
        self.GNI = gather_ni
        self.SP = single_packet
        self.NQ = nq
        # gather preps issued ahead of triggers (descriptor generation for
        # B blocks overlaps phase 1; bounded by SBUF for the gather tiles)
        self.B = prep_ahead
        assert self.IDX_PER_BLK % self.GNI == 0
        self.KSUB = self.GNI // blk         # k-slots filled per gather call
        assert self.NPAD < 2 ** 15          # int16 gather indices
        assert f % 256 == 0 or f == 256


def build(cfg: Cfg):
    """Build + compile the per-core Bass program (identical on all cores)."""
    nc = bacc.Bacc("TRN2", target_bir_lowering=False, debug=False,
                   num_swdge_queues=cfg.NQ)
    F, K, TPS = cfg.F, cfg.K, cfg.TPS
    qctr = [0]

    xt = nc.dram_tensor("xt", [F, cfg.NPAD], BF16, kind="ExternalInput")
    w = nc.dram_tensor("w", [F, F], BF16, kind="ExternalInput")
    bb = nc.dram_tensor("bb", [128, F], F32, kind="ExternalInput")
    ix = nc.dram_tensor("ix", [128, cfg.IDX_COLS], I16, kind="ExternalInput")
    out = nc.dram_tensor("out", [cfg.NPC, F], F32, kind="ExternalOutput")
    # keeps the z-done chain (zchk2) live so DCE can't drop the triggers
    sink = nc.dram_tensor("sink", [1, 16], BF16, kind="ExternalOutput")

    with tile.TileContext(nc) as tc:
        with (
            tc.tile_pool(name="dram", bufs=1, space="DRAM") as dpool,
            tc.tile_pool(name="const", bufs=1) as cpool,
            tc.tile_pool(name="xsl", bufs=2) as xpool,
            tc.tile_pool(name="zsl", bufs=2) as zpool,
            tc.tile_pool(name="ps", bufs=8, space="PSUM") as ppool,
            tc.tile_pool(name="g", bufs=cfg.B) as gpool,
            tc.tile_pool(name="tr", bufs=2) as tpool,
        ):
            z = dpool.tile([cfg.NPAD, F], BF16)

            w0 = cpool.tile([128, F], BF16)
            w1 = cpool.tile([128, F], BF16)
            nc.sync.dma_start(w0, w.ap()[0:128, :])
            nc.sync.dma_start(w1, w.ap()[128:256, :])
            bbc = cpool.tile([128, F], F32)
            nc.sync.dma_start(bbc, bb.ap())
            ixt = cpool.tile([128, cfg.IDX_COLS], I16)
            nc.sync.dma_start(ixt, ix.ap())

            # ---- phase 1: z table (z = x @ W + b, bf16) -------------------
            # Bias is folded into the PSUM->SBUF eviction (tensor_tensor
            # add replaces the plain cast at the same DVE cost), so phase 2
            # reduces to gather + max + relu.
            # eviction: slab tile [p, t, o] -> rows m = (slab*TPS+t)*128 + p
            z_view = z.rearrange("(s t p) o -> p s t o", p=128, t=TPS)
            for slab in range(cfg.NSLABS):
                c0 = slab * TPS * 128
                x0 = xpool.tile([128, TPS * 128], BF16, tag="x0")
                x1 = xpool.tile([128, TPS * 128], BF16, tag="x1")
                nc.sync.dma_start(x0, xt.ap()[0:128, c0:c0 + TPS * 128])
                nc.sync.dma_start(x1, xt.ap()[128:256, c0:c0 + TPS * 128])
                zs = zpool.tile([128, TPS * F], BF16, tag="zs")
                for t in range(TPS):
                    ps = ppool.tile([128, F], F32, tag="ps")
                    nc.tensor.matmul(ps, x0[:, t * 128:(t + 1) * 128], w0,
                                     start=True, stop=False)
                    nc.tensor.matmul(ps, x1[:, t * 128:(t + 1) * 128], w1,
                                     start=False, stop=True)
                    nc.vector.tensor_add(zs[:, t * F:(t + 1) * F], ps, bbc)
                nc.sync.dma_start(z_view[:, slab, :, :],
                                  zs.rearrange("p (t o) -> p t o", t=TPS))

            # ---- phase 2: gather + max-pool + relu -----------------------
            # prepare/trigger split: descriptor GENERATION (the serial
            # ~2.2ns/idx Q7 resource that dominates this kernel) runs as
            # prepare_only preps whose declared in_ap is a 1-row view of z
            # (real row stride via elem_step), so generation only depends on
            # slab 0 and overlaps phase 1.  Orderings Tile can't model are
            # built from real edges:
            #  - z-done: strided z readback -> pool copy (zchk2); every
            #    trigger WAW-writes a zchk2 cell (so drains start after z)
            #  - trigger -> consumer: trigger WAW-writes a cell of the t16
            #    tile (allocated before the trigger)
            #  - data-landed: t16's tensor_tensor carries a hard wait on the
            #    per-queue completion sem baked into the gather descriptors
            gtiles = {}
            # SWDGE sems are locked to one queue, so completion sems are per
            # queue with cumulative waits (each block posts exactly one call
            # per queue, in block order, so block b's data on queue q is the
            # (b+1)-th 16-increment)
            ncalls = cfg.IDX_PER_BLK // cfg.GNI
            assert ncalls == cfg.NQ
            gsems = [nc.alloc_semaphore(f"gsem{q}") for q in range(cfg.NQ)]
            pend = [0] * cfg.NQ

            def emit_prep(b):
                g = gpool.tile([128, K, F], BF16, tag="g")
                for c in range(ncalls):
                    col = (b * cfg.IDX_PER_BLK + c * cfg.GNI) // 16
                    q = c % cfg.NQ
                    nc.gpsimd.dma_gather(
                        out_ap=g[:, c * cfg.KSUB:(c + 1) * cfg.KSUB, :],
                        in_ap=z[0:1, :],
                        elem_step=F,
                        idxs_ap=ixt[:, col:col + cfg.GNI // 16],
                        num_idxs=cfg.GNI,
                        num_idxs_reg=cfg.GNI,
                        elem_size=F,
                        single_packet=cfg.SP,
                        queue_num=q,
                        prepare_only=True,
                        sem=gsems[q],
                    )
                    pend[q] += 1
                gtiles[b] = g

            for b in range(min(cfg.B, cfg.NBLK)):
                emit_prep(b)

            # z-done chain: tiny strided readback touching every slab.
            # Emitted AFTER the prep batch so the pool copy (which waits all
            # evictions) does not sit ahead of the preps in the pool stream.
            zchk = cpool.tile([128, 16], BF16)
            nc.sync.dma_start(
                zchk[0:cfg.NSLABS, :],
                z.rearrange("(s r) o -> s r o", s=cfg.NSLABS)[:, 0, 0:16])
            zchk2 = cpool.tile([128, 16], BF16)
            nc.gpsimd.tensor_copy(zchk2[0:cfg.NSLABS, :],
                                  zchk[0:cfg.NSLABS, :])
            for b in range(cfg.NBLK):
                # stage-1 tiles: s[c] reduces call c's 8 k-slots to 4, so it
                # only needs queue c's data -> exactly one sem wait per op,
                # and trigger c WAW-signals s[c] (model matches reality)
                s = [tpool.tile([128, 4 * F], BF16, tag=f"s{c}",
                                name=f"s{c}_{b}")
                     for c in range(ncalls)]
                for q in range(cfg.NQ):
                    if pend[q]:
                        nc.gpsimd.trigger_dma(
                            count=None, queue_num=q,
                            signals_writable=[zchk2[0:1, 0:1], s[q][0:1, 0:1]])
                        pend[q] = 0
                if b + cfg.B < cfg.NBLK:
                    emit_prep(b + cfg.B)
                g = gtiles.pop(b)
                for c in range(ncalls):
                    k0 = c * cfg.KSUB
                    nc.vector.tensor_tensor(
                        s[c].rearrange("p (k o) -> p k o", k=4),
                        g[:, k0:k0 + 4, :],
                        g[:, k0 + 4:k0 + 8, :],
                        op=mybir.AluOpType.max)._wait_ge(
                        gsems[c], 16 * (b + 1))
                u0 = tpool.tile([128, 4 * F], BF16, tag="u0")
                nc.vector.tensor_tensor(u0, s[0], s[1],
                                        op=mybir.AluOpType.max)
                u1 = tpool.tile([128, 4 * F], BF16, tag="u1")
                nc.vector.tensor_tensor(u1, s[2], s[3],
                                        op=mybir.AluOpType.max)
                t4 = tpool.tile([128, 4 * F], BF16, tag="t4")
                nc.vector.tensor_tensor(t4, u0, u1,
                                        op=mybir.AluOpType.max)
                t2 = tpool.tile([128, 2 * F], BF16, tag="t2")
                nc.vector.tensor_tensor(t2, t4[:, 0:2 * F], t4[:, 2 * F:4 * F],
                                        op=mybir.AluOpType.max)
                t1 = tpool.tile([128, F], F32, tag="t1")
                nc.vector.tensor_tensor(t1, t2[:, 0:F], t2[:, F:2 * F],
                                        op=mybir.AluOpType.max)
                # out-of-place relu on the (idle) scalar engine; the in-place
                # DVE tensor_scalar_max here cost 26us/block (serialized RMW)
                ob = tpool.tile([128, F], F32, tag="ob")
                nc.scalar.activation(ob, t1,
                                     mybir.ActivationFunctionType.Relu)
                rows = min(cfg.BLK, cfg.NPC - b * cfg.BLK)
                nc.sync.dma_start(out.ap()[b * cfg.BLK:b * cfg.BLK + rows, :],
                                  ob[0:rows, :])
            nc.sync.dma_start(sink.ap(), zchk2[0:1, 0:16])

    nc.compile()
    return nc


def prep_inputs(cfg: Cfg, x, neigh, W, b):
    """Host-side input prep. Returns per-core in_maps."""
    bf16 = ml_dtypes.bfloat16
    xt = np.zeros((cfg.F, cfg.NPAD), dtype=bf16)
    xt[:, :cfg.N] = np.ascontiguousarray(x.T).astype(bf16)
    wb = np.ascontiguousarray(W).astype(bf16)
    bbc = np.broadcast_to(np.asarray(b, np.float32)[None, :],
                          (128, cfg.F)).copy()

    # natural z row ids; sort each node's neighbors so the j-th descriptor
    # stripe of every gather hits a narrow quantile window of the table
    r = np.sort(neigh, axis=1).astype(np.int16)            # [N, K]
    in_maps = []
    for c in range(cfg.NCORES):
        rc = r[c * cfg.NPC:(c + 1) * cfg.NPC]               # [NPC, K]
        pad = cfg.NBLK * cfg.BLK - cfg.NPC
        if pad:
            rc = np.concatenate([rc, np.zeros((pad, cfg.K), np.int16)])
        # position i = blk*BLK*K + j*128 + p  must hold r[blk*BLK + p, j]
        seq = rc.reshape(cfg.NBLK, cfg.BLK, cfg.K).transpose(0, 2, 1).ravel()
        wrapped = seq.reshape(-1, 16).T                      # [16, cols]
        ixc = np.ascontiguousarray(np.tile(wrapped, (128 // 16, 1)))
        in_maps.append({"xt": xt, "w": wb, "bb": bbc, "ix": ixc})
    return in_maps


_CACHE = {}


def _get_nc(cfg: Cfg):
    key = (cfg.N, cfg.K, cfg.F, cfg.NCORES, cfg.TPS, cfg.BLK, cfg.GNI,
           cfg.SP, cfg.NQ, cfg.B)
    if key not in _CACHE:
        _CACHE[key] = build(cfg)
    return _CACHE[key]


def kernel(x, neigh, W, b):
    x = np.asarray(x)
    neigh = np.asarray(neigh)
    W = np.asarray(W)
    b = np.asarray(b)
    cfg = Cfg(n=x.shape[0], k=neigh.shape[1], f=W.shape[0])
    nc = _get_nc(cfg)
    in_maps = prep_inputs(cfg, x, neigh, W, b)
    res = bass_utils.run_bass_kernel_spmd(nc, in_maps,
                                          core_ids=list(range(cfg.NCORES)))
    return np.concatenate([res.results[c]["out"] for c in range(cfg.NCORES)],
                          axis=0)

